# revision 15
# baseline (speedup 1.0000x reference)
"""Trainium2 Bass kernel for causal MHA + RoPE (B=1, S=2048, E=2048, H=16, D=128).

Sharding: tensor-parallel over heads. Each of 8 cores owns 2 heads:
  - Wq/Wk/Wv column-sharded (each core computes its 256 q/k/v features),
  - Wo row-sharded (each core produces a partial [S, E] output),
  - partials summed on host (the "all-reduce").

Per-core device pipeline (all matmuls bf16 operands, fp32 PSUM accumulation):
  1. Q^T = Wq_c @ x^T, K^T = Wk_c @ x^T   (layout [d, s], d on partitions)
     V    = x @ Wv_c^T                    (layout [s, d])
  2. RoPE on Q^T/K^T via DVE (sign-folded sin table prepared on host).
  3. Per (head, q-tile of 512): scores^T[k,q] blocks of [128, 512] via PE,
     exp on ScalarE (PSUM->SBUF, bf16 out), causal mask multiply on the 4
     diagonal blocks only (fully-masked blocks are skipped entirely),
     denominator row via ones-matmul, attention out^T accum via PE,
     normalize with reciprocal broadcast (K=1 fp32r matmul broadcast).
  4. out_partial = attn @ Wo_c^T directly from attn^T (no transposes anywhere).
"""

import math

import numpy as np
import ml_dtypes

import concourse.bass as bass
import concourse.mybir as mybir
import concourse.tile as tile
from concourse.bass_utils import run_bass_kernel_spmd

BF16 = mybir.dt.bfloat16
F32 = mybir.dt.float32
F32R = mybir.dt.float32r
AF = mybir.ActivationFunctionType

S = 2048
E = 2048
D = 128          # head dim
NCORES = 8
HPC = 2          # heads per core
F2 = HPC * D     # 256 per-core qkv features
EC = E // 128    # 16 contraction chunks
NQ = 512         # query tile width
NJ = S // NQ     # 4 query tiles
NKB = S // 128   # 16 key blocks


def build_nc() -> bass.Bass:
    nc = bass.Bass()

    xT = nc.dram_tensor("xT", [E, S], BF16, kind="ExternalInput")
    wq = nc.dram_tensor("wq", [E, F2], BF16, kind="ExternalInput")
    wk = nc.dram_tensor("wk", [E, F2], BF16, kind="ExternalInput")
    wv = nc.dram_tensor("wv", [E, F2], BF16, kind="ExternalInput")
    wo = nc.dram_tensor("wo", [F2, E], BF16, kind="ExternalInput")
    cosT = nc.dram_tensor("cosT", [D, S], F32, kind="ExternalInput")
    sinS = nc.dram_tensor("sinS", [D, S], F32, kind="ExternalInput")
    masks = nc.dram_tensor("masks", [4, 128, NQ], BF16, kind="ExternalInput")
    out = nc.dram_tensor("out", [S, E], F32, kind="ExternalOutput")

    with tile.TileContext(nc) as tc:
        _emit(nc, tc, xT, wq, wk, wv, wo, cosT, sinS, masks, out)
    _split_multi_waits(nc)
    return nc


def _split_multi_waits(nc):
    """Walrus codegen only allows ONE sync-wait per TPB instruction (the
    "Too many sync wait commands" error). Tile sometimes attaches several.
    Split: insert wait-only EventSemaphore nops (one wait each) before the
    offending instruction on the same engine."""
    nsplit = 0
    for fn in nc.m.functions:
        for blk in fn.blocks:
            out_insts = []
            for inst in blk.instructions:
                si = inst.sync_info
                if si is not None and si.on_wait and len(si.on_wait) > 1:
                    waits = list(si.on_wait)
                    for k, w in enumerate(waits[:-1]):
                        ev = mybir.InstEventSemaphore(name=f"{inst.name}-ws{k}")
                        ev.engine = inst.engine
                        ev.sync_info = mybir.SyncInfo(on_wait=[w], on_update=[])
                        out_insts.append(ev)
                        nsplit += 1
                    inst.sync_info = mybir.SyncInfo(
                        on_wait=[waits[-1]], on_update=list(si.on_update or [])
                    )
                out_insts.append(inst)
            blk.instructions = out_insts
    return nsplit


import os
PS_CFG = tuple(int(x) for x in os.environ.get("PS_CFG", "2,2,1,1,2").split(","))


def _emit(nc, tc, xT, wq, wk, wv, wo, cosT, sinS, masks, out):
    from contextlib import ExitStack

    a, b, d, c, e = PS_CFG
    with ExitStack() as ctx:
        consts = ctx.enter_context(tc.tile_pool(name="consts", bufs=1))
        state = ctx.enter_context(tc.tile_pool(name="state", bufs=1))
        tmps = ctx.enter_context(tc.tile_pool(name="tmps", bufs=2))
        psA = ctx.enter_context(tc.tile_pool(name="psA", bufs=a, space="PSUM"))
        psB = ctx.enter_context(tc.tile_pool(name="psB", bufs=b, space="PSUM"))
        psD = ctx.enter_context(tc.tile_pool(name="psD", bufs=d, space="PSUM"))
        psC = ctx.enter_context(tc.tile_pool(name="psC", bufs=c, space="PSUM"))
        psE = ctx.enter_context(tc.tile_pool(name="psE", bufs=e, space="PSUM"))

        # ---- constants / weights to SBUF ----
        wq_sb = consts.tile([128, EC, F2], BF16)
        nc.sync.dma_start(wq_sb, wq.rearrange("(c p) f -> p c f", p=128))
        wk_sb = consts.tile([128, EC, F2], BF16)
        nc.sync.dma_start(wk_sb, wk.rearrange("(c p) f -> p c f", p=128))
        wv_sb = consts.tile([128, EC, F2], BF16)
        nc.sync.dma_start(wv_sb, wv.rearrange("(c p) f -> p c f", p=128))
        wo_sb = consts.tile([128, HPC, E], BF16)
        nc.sync.dma_start(wo_sb, wo.rearrange("(c p) e -> p c e", p=128))
        cos_sb = consts.tile([D, S], F32)
        nc.sync.dma_start(cos_sb, cosT[:, :])
        sinS_sb = consts.tile([D, S], F32)
        nc.sync.dma_start(sinS_sb, sinS[:, :])
        masks_sb = consts.tile([128, 4, NQ], BF16)
        nc.sync.dma_start(masks_sb, masks.rearrange("c p q -> p c q"))
        ones_col = consts.tile([128, 1], BF16)
        nc.vector.memset(ones_col, 1.0)
        ones_colb = consts.tile([1, 128], BF16)
        nc.vector.memset(ones_colb, 1.0)

        # Absorb const-DMA waits into DVE's vector clock early: DVE
        # TensorTensor instructions only have ONE sync-wait slot, so ops
        # reading a const AND a PSUM tile must not need a DMA wait too.
        absorb = consts.tile([1, 4], F32)
        nc.vector.tensor_copy(absorb[0:1, 0:1], cos_sb[0:1, 0:1])
        nc.vector.tensor_copy(absorb[0:1, 1:2], sinS_sb[0:1, 0:1])
        nc.vector.tensor_copy(absorb[0:1, 2:3], masks_sb[0:1, 0, 0:1])
        # rotating scratch for single-wait carrier copies (see _carrier)
        carrier_sb = consts.tile([1, 64], F32)
        carrier_act_sb = consts.tile([1, 64], F32)
        carrier_i = [0]

        def carrier_act(src_ap):
            """ACT-engine wait absorber (same idea as carrier, on ScalarE)."""
            i = carrier_i[0] % 64
            carrier_i[0] += 1
            nc.scalar.copy(carrier_act_sb[0:1, i:i + 1], src_ap)

        def carrier(src_ap):
            """Tiny DVE op that absorbs one semaphore wait (e.g. a PSUM RAW
            on PE) into DVE's vector clock, so the next real DVE op needs at
            most one other wait (TensorTensor has a single wait slot)."""
            i = carrier_i[0] % 64
            carrier_i[0] += 1
            nc.vector.tensor_copy(carrier_sb[0:1, i:i + 1], src_ap)

        QrT = state.tile([D, HPC, S], BF16)
        KrT = state.tile([D, HPC, S], BF16)
        V_sb = state.tile([128, NKB, F2], BF16)
        attnT = state.tile([D, HPC, S], BF16)
        eP = state.tile([128, NKB, NQ], BF16)
        ost_ring = state.tile([128, 4, NQ], F32)
        ost_i = [0]

        if True:
            xT_sb = state.tile([128, EC, S], BF16)
            xr = xT.rearrange("(c p) s -> p c s", p=128)
            for s4 in range(NJ):
                for e in range(EC):
                    nc.sync.dma_start(
                        xT_sb[:, e, s4 * NQ:(s4 + 1) * NQ],
                        xr[:, e, s4 * NQ:(s4 + 1) * NQ],
                    )

            # ---- QKV projections, interleaved per s-chunk ----
            def qk_group(w_sb, dstT, f, s4):
                sl = slice(s4 * NQ, (s4 + 1) * NQ)
                ps = psA.tile([128, NQ], F32, tag="A", name="ps_proj")
                for e in range(EC):
                    nc.tensor.matmul(
                        ps,
                        lhsT=w_sb[:, e, f * 128:(f + 1) * 128],
                        rhs=xT_sb[:, e, sl],
                        start=(e == 0),
                        stop=(e == EC - 1),
                    )
                t1 = tmps.tile([128, NQ], F32, tag="ropeA", name="t1")
                t2 = tmps.tile([128, NQ], F32, tag="ropeB", name="t2")
                carrier(ps[0:1, 0:1])
                nc.vector.tensor_mul(t1[0:64, :], ps[64:128, :], sinS_sb[0:64, sl])
                nc.vector.tensor_mul(t1[64:128, :], ps[0:64, :], sinS_sb[64:128, sl])
                nc.vector.tensor_mul(t2, ps, cos_sb[:, sl])
                nc.vector.tensor_add(dstT[:, f, sl], t1, t2)

            def v_group(sc):
                psv = psB.tile([128, F2], F32, tag="B", name="ps_v")
                for e in range(EC):
                    nc.tensor.matmul(
                        psv,
                        lhsT=xT_sb[:, e, sc * 128:(sc + 1) * 128],
                        rhs=wv_sb[:, e, :],
                        start=(e == 0),
                        stop=(e == EC - 1),
                    )
                nc.scalar.copy(V_sb[:, sc, :], psv)

            for s4 in range(NJ):
                qk_group(wq_sb, QrT, 0, s4)
                qk_group(wk_sb, KrT, 0, s4)
                v_group(4 * s4 + 0)
                v_group(4 * s4 + 1)
                qk_group(wq_sb, QrT, 1, s4)
                qk_group(wk_sb, KrT, 1, s4)
                v_group(4 * s4 + 2)
                v_group(4 * s4 + 3)

        # ---- attention + out-proj ----
        if True:
            for j in range(NJ):
                qsl = slice(j * NQ, (j + 1) * NQ)
                nblk = 4 * (j + 1)
                for h in range(HPC):
                    ps_d = psD.tile([1, NQ], F32, tag="D", name="ps_d")
                    ps_o = psB.tile([128, NQ], F32, tag="B", name="ps_o")

                    def scores(kb):
                        ps_s = psA.tile([128, NQ], F32, tag="A", name="ps_s")
                        nc.tensor.matmul(
                            ps_s,
                            lhsT=KrT[:, h, kb * 128:(kb + 1) * 128],
                            rhs=QrT[:, h, qsl],
                            start=True,
                            stop=True,
                        )
                        carrier_act(eP[0:1, kb, 0:1])
                        nc.scalar.activation(eP[:, kb, :], ps_s, AF.Exp)
                        if kb >= nblk - 4:
                            nc.vector.tensor_mul(
                                eP[:, kb, :], eP[:, kb, :],
                                masks_sb[:, kb - (nblk - 4), :],
                            )

                    def accum(kb):
                        nc.tensor.matmul(
                            ps_d, lhsT=ones_col, rhs=eP[:, kb, :],
                            start=(kb == 0), stop=(kb == nblk - 1),
                        )
                        nc.tensor.matmul(
                            ps_o,
                            lhsT=V_sb[:, kb, h * 128:(h + 1) * 128],
                            rhs=eP[:, kb, :],
                            start=(kb == 0), stop=(kb == nblk - 1),
                        )

                    # software-pipeline depth 2: scores(kb+2) issued
                    # before accum(kb) so exp latency is hidden
                    scores(0)
                    scores(1)
                    for kb in range(2, nblk):
                        scores(kb)
                        accum(kb - 2)
                    accum(nblk - 2)
                    accum(nblk - 1)

                    rec = tmps.tile([1, NQ], F32, tag="rec", name="rec")
                    nc.vector.reciprocal(rec, ps_d)
                    rec_hi = tmps.tile([1, NQ], BF16, tag="rech", name="rec_hi")
                    nc.vector.tensor_copy(rec_hi, rec)
                    rec_lo = tmps.tile([1, NQ], BF16, tag="recl", name="rec_lo")
                    nc.vector.tensor_sub(rec_lo, rec, rec_hi)
                    ps_b = psC.tile([128, NQ], F32, tag="C", name="ps_b")
                    nc.tensor.matmul(ps_b, lhsT=ones_colb, rhs=rec_hi, start=True, stop=False)
                    nc.tensor.matmul(ps_b, lhsT=ones_colb, rhs=rec_lo, start=False, stop=True)
                    bc = tmps.tile([128, NQ], F32, tag="bc", name="bc")
                    nc.vector.tensor_copy(bc, ps_b)
                    nc.vector.tensor_mul(attnT[:, h, qsl], ps_o, bc)

                # out-proj for this q-tile (both heads ready)
                for sc in range(4 * j, 4 * j + 4):
                    for ec in range(4):
                        pso = psE.tile([128, NQ], F32, tag="E", name="pso")
                        for hc in range(HPC):
                            nc.tensor.matmul(
                                pso,
                                lhsT=attnT[:, hc, sc * 128:(sc + 1) * 128],
                                rhs=wo_sb[:, hc, ec * NQ:(ec + 1) * NQ],
                                start=(hc == 0),
                                stop=(hc == HPC - 1),
                            )
                        oi = ost_i[0] % 4
                        ost_i[0] += 1
                        ost = ost_ring[:, oi, :]
                        # in-place 1-elem write first: absorbs the WAR-vs-DMA
                        # tick so the real copy only carries the PE wait
                        nc.vector.tensor_copy(ost[0:1, 0:1], ost[0:1, 0:1])
                        nc.vector.tensor_copy(ost, pso)
                        nc.sync.dma_start(
                            out[sc * 128:(sc + 1) * 128, ec * NQ:(ec + 1) * NQ], ost
                        )


_NC_CACHE = None


def _get_nc():
    global _NC_CACHE
    if _NC_CACHE is None:
        _NC_CACHE = build_nc()
    return _NC_CACHE


def _prep_inputs(x, rotary_cos, rotary_sin, Wq, Wk, Wv, Wo):
    bf = ml_dtypes.bfloat16
    x = np.asarray(x, dtype=np.float32)
    Wq = np.asarray(Wq, dtype=np.float32)
    Wk = np.asarray(Wk, dtype=np.float32)
    Wv = np.asarray(Wv, dtype=np.float32)
    Wo = np.asarray(Wo, dtype=np.float32)
    cos = np.asarray(rotary_cos, dtype=np.float32)[0]  # [S, D]
    sin = np.asarray(rotary_sin, dtype=np.float32)[0]

    xT = np.ascontiguousarray(x[0].T).astype(bf)          # [E, S]
    cosT = np.ascontiguousarray(cos.T)                     # [D, S]
    sinT = cos.T * 0 + sin.T
    sinS = np.concatenate([-sinT[:64], sinT[64:]], axis=0)
    sinS = np.ascontiguousarray(sinS.astype(np.float32))

    # 4 diagonal-mask tiles: mask[idx, k, q] = 1 if k + 128*idx <= q
    kk = np.arange(128)[:, None]
    qq = np.arange(NQ)[None, :]
    m = np.stack([(kk + 128 * i <= qq) for i in range(4)]).astype(bf)
    masks = np.ascontiguousarray(m)

    scale = 1.0 / math.sqrt(D)
    in_maps = []
    for c in range(NCORES):
        fs = slice(F2 * c, F2 * (c + 1))
        in_maps.append({
            "xT": xT,
            "wq": np.ascontiguousarray((Wq[fs, :] * scale).T).astype(bf),
            "wk": np.ascontiguousarray(Wk[fs, :].T).astype(bf),
            "wv": np.ascontiguousarray(Wv[fs, :].T).astype(bf),
            "wo": np.ascontiguousarray(Wo[:, fs].T).astype(bf),
            "cosT": cosT,
            "sinS": sinS,
            "masks": masks,
        })
    return in_maps


def kernel(x, rotary_cos, rotary_sin, Wq, Wk, Wv, Wo, **run_kwargs):
    nc = _get_nc()
    in_maps = _prep_inputs(x, rotary_cos, rotary_sin, Wq, Wk, Wv, Wo)
    res = run_bass_kernel_spmd(nc, in_maps, core_ids=list(range(NCORES)), **run_kwargs)
    acc = np.zeros((S, E), dtype=np.float64)
    for r in res.results:
        acc += r["out"].astype(np.float64)
    full = acc.astype(np.float32).reshape(1, S, E)
    if run_kwargs:
        return full, res
    return full


# revision 26
# speedup vs baseline: 1.0498x; 1.0498x over previous
"""Trainium2 Bass kernel for causal MHA + RoPE (B=1, S=2048, E=2048, H=16, D=128).

Sharding: tensor-parallel over heads. Each of 8 cores owns 2 heads:
  - Wq/Wk/Wv column-sharded (each core computes its 256 q/k/v features),
  - Wo row-sharded (each core produces a partial [S, E] output),
  - partials summed on host (the "all-reduce").

Per-core device pipeline (all matmuls bf16 operands, fp32 PSUM accumulation):
  1. Q^T = Wq_c @ x^T, K^T = Wk_c @ x^T   (layout [d, s], d on partitions)
     V    = x @ Wv_c^T                    (layout [s, d])
  2. RoPE on Q^T/K^T via DVE (sign-folded sin table prepared on host).
  3. Per (head, q-tile of 512): scores^T[k,q] blocks of [128, 512] via PE,
     exp on ScalarE (PSUM->SBUF, bf16 out), causal mask multiply on the 4
     diagonal blocks only (fully-masked blocks are skipped entirely),
     denominator row via ones-matmul, attention out^T accum via PE,
     normalize with reciprocal broadcast (K=1 fp32r matmul broadcast).
  4. out_partial = attn @ Wo_c^T directly from attn^T (no transposes anywhere).
"""

import math

import numpy as np
import ml_dtypes

import concourse.bass as bass
import concourse.mybir as mybir
import concourse.tile as tile
from concourse.bass_utils import run_bass_kernel_spmd

BF16 = mybir.dt.bfloat16
F32 = mybir.dt.float32
F32R = mybir.dt.float32r
AF = mybir.ActivationFunctionType

S = 2048
E = 2048
D = 128          # head dim
NCORES = 8
HPC = 2          # heads per core
F2 = HPC * D     # 256 per-core qkv features
EC = E // 128    # 16 contraction chunks
NQ = 512         # query tile width
NJ = S // NQ     # 4 query tiles
NKB = S // 128   # 16 key blocks


def build_nc() -> bass.Bass:
    nc = bass.Bass()

    xT = nc.dram_tensor("xT", [E, S], BF16, kind="ExternalInput")
    wq = nc.dram_tensor("wq", [E, F2], BF16, kind="ExternalInput")
    wk = nc.dram_tensor("wk", [E, F2], BF16, kind="ExternalInput")
    wv = nc.dram_tensor("wv", [E, F2], BF16, kind="ExternalInput")
    wo = nc.dram_tensor("wo", [F2, E], BF16, kind="ExternalInput")
    cosT = nc.dram_tensor("cosT", [D, S], F32, kind="ExternalInput")
    sinS = nc.dram_tensor("sinS", [D, S], F32, kind="ExternalInput")
    masks = nc.dram_tensor("masks", [4, 128, NQ], BF16, kind="ExternalInput")
    out = nc.dram_tensor("out", [S, E], BF16, kind="ExternalOutput")

    with tile.TileContext(nc) as tc:
        _emit(nc, tc, xT, wq, wk, wv, wo, cosT, sinS, masks, out)
    _split_multi_waits(nc)
    return nc


def _split_multi_waits(nc):
    """Walrus codegen only allows ONE sync-wait per TPB instruction (the
    "Too many sync wait commands" error). Tile sometimes attaches several.
    Split: insert wait-only EventSemaphore nops (one wait each) before the
    offending instruction on the same engine."""
    nsplit = 0
    for fn in nc.m.functions:
        for blk in fn.blocks:
            out_insts = []
            for inst in blk.instructions:
                si = inst.sync_info
                if si is not None and si.on_wait and len(si.on_wait) > 1:
                    waits = list(si.on_wait)
                    for k, w in enumerate(waits[:-1]):
                        ev = mybir.InstEventSemaphore(name=f"{inst.name}-ws{k}")
                        ev.engine = inst.engine
                        ev.sync_info = mybir.SyncInfo(on_wait=[w], on_update=[])
                        out_insts.append(ev)
                        nsplit += 1
                    inst.sync_info = mybir.SyncInfo(
                        on_wait=[waits[-1]], on_update=list(si.on_update or [])
                    )
                out_insts.append(inst)
            blk.instructions = out_insts
    return nsplit


PS_CFG = (2, 2, 1, 1, 3)  # PSUM pool bufs: scores, PV/V, denom, bcast, outproj


def _emit(nc, tc, xT, wq, wk, wv, wo, cosT, sinS, masks, out):
    from contextlib import ExitStack

    a, b, d, c, e = PS_CFG
    with ExitStack() as ctx:
        consts = ctx.enter_context(tc.tile_pool(name="consts", bufs=1))
        state = ctx.enter_context(tc.tile_pool(name="state", bufs=1))
        tmps = ctx.enter_context(tc.tile_pool(name="tmps", bufs=2))
        psA = ctx.enter_context(tc.tile_pool(name="psA", bufs=a, space="PSUM"))
        psB = ctx.enter_context(tc.tile_pool(name="psB", bufs=b, space="PSUM"))
        psD = ctx.enter_context(tc.tile_pool(name="psD", bufs=d, space="PSUM"))
        psC = ctx.enter_context(tc.tile_pool(name="psC", bufs=c, space="PSUM"))
        psE = ctx.enter_context(tc.tile_pool(name="psE", bufs=e, space="PSUM"))

        # ---- constants / weights to SBUF (DMA order = need order) ----
        wq_sb = consts.tile([128, EC, F2], BF16)
        wk_sb = consts.tile([128, EC, F2], BF16)
        wv_sb = consts.tile([128, EC, F2], BF16)
        wo_sb = consts.tile([128, HPC, E], BF16)
        cos_sb = consts.tile([D, S], F32)
        sinS_sb = consts.tile([D, S], F32)
        masks_sb = consts.tile([128, 4, NQ], BF16)
        wqr = wq.rearrange("(c p) f -> p c f", p=128)
        nc.sync.dma_start(cos_sb, cosT[:, :])
        nc.sync.dma_start(sinS_sb, sinS[:, :])
        ones_col = consts.tile([128, 1], BF16)
        nc.vector.memset(ones_col, 1.0)
        ones_colb = consts.tile([1, 128], BF16)
        nc.vector.memset(ones_colb, 1.0)

        # Absorb const-DMA waits into DVE's vector clock early: DVE
        # TensorTensor instructions only have ONE sync-wait slot, so ops
        # reading a const AND a PSUM tile must not need a DMA wait too.
        absorb = consts.tile([1, 4], F32)
        nc.vector.tensor_copy(absorb[0:1, 0:1], cos_sb[0:1, 0:1])
        nc.vector.tensor_copy(absorb[0:1, 1:2], sinS_sb[0:1, 0:1])
        nc.vector.tensor_copy(absorb[0:1, 2:3], masks_sb[0:1, 0, 0:1])
        # rotating scratch for single-wait carrier copies (see _carrier)
        carrier_sb = consts.tile([1, 64], F32)
        carrier_act_sb = consts.tile([1, 64], F32)
        carrier_i = [0]

        def carrier_act(src_ap):
            """ACT-engine wait absorber (same idea as carrier, on ScalarE)."""
            i = carrier_i[0] % 64
            carrier_i[0] += 1
            nc.scalar.copy(carrier_act_sb[0:1, i:i + 1], src_ap)

        def carrier(src_ap):
            """Tiny DVE op that absorbs one semaphore wait (e.g. a PSUM RAW
            on PE) into DVE's vector clock, so the next real DVE op needs at
            most one other wait (TensorTensor has a single wait slot)."""
            i = carrier_i[0] % 64
            carrier_i[0] += 1
            nc.vector.tensor_copy(carrier_sb[0:1, i:i + 1], src_ap)

        QrT = state.tile([D, HPC, S], BF16)
        KrT = state.tile([D, HPC, S], BF16)
        V_sb = state.tile([128, NKB, F2], BF16)
        attnT = state.tile([D, HPC, S], BF16)
        eP = state.tile([128, NKB, NQ], BF16)
        ost_ring = state.tile([128, 6, NQ], BF16)
        ost_i = [0]

        if True:
            xT_sb = state.tile([128, EC, S], BF16)
            xr = xT.rearrange("(c p) s -> p c s", p=128)

            def dma_x(s4):
                for e in range(EC):
                    nc.sync.dma_start(
                        xT_sb[:, e, s4 * NQ:(s4 + 1) * NQ],
                        xr[:, e, s4 * NQ:(s4 + 1) * NQ],
                    )
            for e in range(EC):
                nc.sync.dma_start(wq_sb[:, e, :], wqr[:, e, :])
                nc.sync.dma_start(
                    xT_sb[:, e, 0:NQ],
                    xr[:, e, 0:NQ],
                )
            nc.sync.dma_start(wk_sb, wk.rearrange("(c p) f -> p c f", p=128))
            nc.sync.dma_start(wv_sb, wv.rearrange("(c p) f -> p c f", p=128))
            dma_x(1)
            nc.sync.dma_start(masks_sb, masks.rearrange("c p q -> p c q"))
            nc.sync.dma_start(wo_sb, wo.rearrange("(c p) e -> p c e", p=128))
            dma_x(2)
            dma_x(3)

            # ---- QKV projections, interleaved per s-chunk ----
            def qk_group(w_sb, dstT, f, s4):
                sl = slice(s4 * NQ, (s4 + 1) * NQ)
                ps = psA.tile([128, NQ], F32, tag="A", name="ps_proj")
                for e in range(EC):
                    nc.tensor.matmul(
                        ps,
                        lhsT=w_sb[:, e, f * 128:(f + 1) * 128],
                        rhs=xT_sb[:, e, sl],
                        start=(e == 0),
                        stop=(e == EC - 1),
                    )
                t1 = tmps.tile([128, NQ], F32, tag="ropeA", name="t1")
                t2 = tmps.tile([128, NQ], F32, tag="ropeB", name="t2")
                carrier(ps[0:1, 0:1])
                nc.vector.tensor_mul(t1[0:64, :], ps[64:128, :], sinS_sb[0:64, sl])
                nc.vector.tensor_mul(t1[64:128, :], ps[0:64, :], sinS_sb[64:128, sl])
                nc.vector.tensor_mul(t2, ps, cos_sb[:, sl])
                nc.vector.tensor_add(dstT[:, f, sl], t1, t2)

            def v_group(sc):
                psv = psB.tile([128, F2], F32, tag="B", name="ps_v")
                for e in range(EC):
                    nc.tensor.matmul(
                        psv,
                        lhsT=xT_sb[:, e, sc * 128:(sc + 1) * 128],
                        rhs=wv_sb[:, e, :],
                        start=(e == 0),
                        stop=(e == EC - 1),
                    )
                nc.scalar.copy(V_sb[:, sc, :], psv)

            for s4 in range(NJ):
                qk_group(wq_sb, QrT, 0, s4)
                qk_group(wk_sb, KrT, 0, s4)
                v_group(4 * s4 + 0)
                v_group(4 * s4 + 1)
                qk_group(wq_sb, QrT, 1, s4)
                qk_group(wk_sb, KrT, 1, s4)
                v_group(4 * s4 + 2)
                v_group(4 * s4 + 3)

        # ---- attention + out-proj ----
        if True:
            for j in range(NJ):
                qsl = slice(j * NQ, (j + 1) * NQ)
                nblk = 4 * (j + 1)
                for h in range(HPC):
                    ps_d = psD.tile([1, NQ], F32, tag="D", name="ps_d")
                    ps_o = psB.tile([128, NQ], F32, tag="B", name="ps_o")

                    def scores(kb):
                        ps_s = psA.tile([128, NQ], F32, tag="A", name="ps_s")
                        nc.tensor.matmul(
                            ps_s,
                            lhsT=KrT[:, h, kb * 128:(kb + 1) * 128],
                            rhs=QrT[:, h, qsl],
                            start=True,
                            stop=True,
                        )
                        carrier_act(eP[0:1, kb, 0:1])
                        nc.scalar.activation(eP[:, kb, :], ps_s, AF.Exp)
                        if kb >= nblk - 4:
                            nc.vector.tensor_mul(
                                eP[:, kb, :], eP[:, kb, :],
                                masks_sb[:, kb - (nblk - 4), :],
                            )

                    def accum(kb):
                        nc.tensor.matmul(
                            ps_d, lhsT=ones_col, rhs=eP[:, kb, :],
                            start=(kb == 0), stop=(kb == nblk - 1),
                        )
                        nc.tensor.matmul(
                            ps_o,
                            lhsT=V_sb[:, kb, h * 128:(h + 1) * 128],
                            rhs=eP[:, kb, :],
                            start=(kb == 0), stop=(kb == nblk - 1),
                        )

                    # software-pipeline depth 2: scores(kb+2) issued
                    # before accum(kb) so exp latency is hidden
                    # software-pipeline depth 2: scores(kb+2) issued
                    # before accum(kb) so exp latency is hidden
                    scores(0)
                    scores(1)
                    for kb in range(2, nblk):
                        scores(kb)
                        accum(kb - 2)
                    accum(nblk - 2)
                    accum(nblk - 1)

                    rec = tmps.tile([1, NQ], F32, tag="rec", name="rec")
                    nc.vector.reciprocal(rec, ps_d)
                    bc = tmps.tile([128, NQ], F32, tag="bc", name="bc")
                    rec_bcast = bass.AP(
                        tensor=rec.tensor,
                        offset=rec.offset,
                        ap=[[0, 128]] + [list(p) for p in rec.ap[1:]],
                    )
                    nc.gpsimd.dma_start(out=bc, in_=rec_bcast)
                    nc.vector.tensor_mul(attnT[:, h, qsl], ps_o, bc)

                # out-proj for this q-tile (both heads ready)
                for sc in range(4 * j, 4 * j + 4):
                    for ec in range(4):
                        pso = psE.tile([128, NQ], F32, tag="E", name="pso")
                        for hc in range(HPC):
                            nc.tensor.matmul(
                                pso,
                                lhsT=attnT[:, hc, sc * 128:(sc + 1) * 128],
                                rhs=wo_sb[:, hc, ec * NQ:(ec + 1) * NQ],
                                start=(hc == 0),
                                stop=(hc == HPC - 1),
                            )
                        oi = ost_i[0] % 6
                        ost_i[0] += 1
                        ost = ost_ring[:, oi, :]
                        # alternate the PSUM->SBUF copy between DVE and ACT
                        # so copy throughput is not the outproj bottleneck
                        if oi % 2 == 0:
                            nc.vector.tensor_copy(ost[0:1, 0:1], ost[0:1, 0:1])
                            nc.vector.tensor_copy(ost, pso)
                        else:
                            nc.scalar.copy(ost[0:1, 0:1], ost[0:1, 0:1])
                            nc.scalar.copy(ost, pso)
                        nc.sync.dma_start(
                            out[sc * 128:(sc + 1) * 128, ec * NQ:(ec + 1) * NQ], ost
                        )


_NC_CACHE = None


def _get_nc():
    global _NC_CACHE
    if _NC_CACHE is None:
        _NC_CACHE = build_nc()
    return _NC_CACHE


def _prep_inputs(x, rotary_cos, rotary_sin, Wq, Wk, Wv, Wo):
    bf = ml_dtypes.bfloat16
    x = np.asarray(x, dtype=np.float32)
    Wq = np.asarray(Wq, dtype=np.float32)
    Wk = np.asarray(Wk, dtype=np.float32)
    Wv = np.asarray(Wv, dtype=np.float32)
    Wo = np.asarray(Wo, dtype=np.float32)
    cos = np.asarray(rotary_cos, dtype=np.float32)[0]  # [S, D]
    sin = np.asarray(rotary_sin, dtype=np.float32)[0]

    xT = np.ascontiguousarray(x[0].T).astype(bf)          # [E, S]
    cosT = np.ascontiguousarray(cos.T)                     # [D, S]
    sinT = cos.T * 0 + sin.T
    sinS = np.concatenate([-sinT[:64], sinT[64:]], axis=0)
    sinS = np.ascontiguousarray(sinS.astype(np.float32))

    # 4 diagonal-mask tiles: mask[idx, k, q] = 1 if k + 128*idx <= q
    kk = np.arange(128)[:, None]
    qq = np.arange(NQ)[None, :]
    m = np.stack([(kk + 128 * i <= qq) for i in range(4)]).astype(bf)
    masks = np.ascontiguousarray(m)

    scale = 1.0 / math.sqrt(D)
    in_maps = []
    for c in range(NCORES):
        fs = slice(F2 * c, F2 * (c + 1))
        in_maps.append({
            "xT": xT,
            "wq": np.ascontiguousarray((Wq[fs, :] * scale).T).astype(bf),
            "wk": np.ascontiguousarray(Wk[fs, :].T).astype(bf),
            "wv": np.ascontiguousarray(Wv[fs, :].T).astype(bf),
            "wo": np.ascontiguousarray(Wo[:, fs].T).astype(bf),
            "cosT": cosT,
            "sinS": sinS,
            "masks": masks,
        })
    return in_maps


def kernel(x, rotary_cos, rotary_sin, Wq, Wk, Wv, Wo, **run_kwargs):
    nc = _get_nc()
    in_maps = _prep_inputs(x, rotary_cos, rotary_sin, Wq, Wk, Wv, Wo)
    res = run_bass_kernel_spmd(nc, in_maps, core_ids=list(range(NCORES)), **run_kwargs)
    acc = np.zeros((S, E), dtype=np.float64)
    for r in res.results:
        acc += r["out"].astype(np.float64)
    full = acc.astype(np.float32).reshape(1, S, E)
    if run_kwargs:
        return full, res
    return full


# revision 35
# speedup vs baseline: 1.0968x; 1.0448x over previous
"""Trainium2 Bass kernel for causal MHA + RoPE (B=1, S=2048, E=2048, H=16, D=128).

Sharding: tensor-parallel over heads. Each of 8 cores owns 2 heads:
  - Wq/Wk/Wv column-sharded (each core computes its 256 q/k/v features),
  - Wo row-sharded (each core produces a partial [S, E] output),
  - partials summed on host (the "all-reduce").

Per-core device pipeline (all matmuls bf16 operands, fp32 PSUM accumulation):
  1. Q^T = Wq_c @ x^T, K^T = Wk_c @ x^T   (layout [d, s], d on partitions)
     V    = x @ Wv_c^T                    (layout [s, d])
  2. RoPE on Q^T/K^T via DVE (sign-folded sin table prepared on host).
  3. Per (head, q-tile of 512): scores^T[k,q] blocks of [128, 512] via PE,
     exp on ScalarE (PSUM->SBUF, bf16 out), causal mask multiply on the 4
     diagonal blocks only (fully-masked blocks are skipped entirely),
     denominator row via ones-matmul, attention out^T accum via PE,
     normalize with reciprocal broadcast (K=1 fp32r matmul broadcast).
  4. out_partial = attn @ Wo_c^T directly from attn^T (no transposes anywhere).
"""

import math

import numpy as np
import ml_dtypes

import concourse.bass as bass
import concourse.mybir as mybir
import concourse.tile as tile
from concourse.bass_utils import run_bass_kernel_spmd

BF16 = mybir.dt.bfloat16
F32 = mybir.dt.float32
F32R = mybir.dt.float32r
AF = mybir.ActivationFunctionType

S = 2048
E = 2048
D = 128          # head dim
NCORES = 8
HPC = 2          # heads per core
F2 = HPC * D     # 256 per-core qkv features
EC = E // 128    # 16 contraction chunks
NQ = 512         # query tile width
NJ = S // NQ     # 4 query tiles
NKB = S // 128   # 16 key blocks


def build_nc() -> bass.Bass:
    nc = bass.Bass()

    xT = nc.dram_tensor("xT", [E, S], BF16, kind="ExternalInput")
    wq = nc.dram_tensor("wq", [E, F2], BF16, kind="ExternalInput")
    wk = nc.dram_tensor("wk", [E, F2], BF16, kind="ExternalInput")
    wv = nc.dram_tensor("wv", [E, F2], BF16, kind="ExternalInput")
    wo = nc.dram_tensor("wo", [F2, E], BF16, kind="ExternalInput")
    cosT = nc.dram_tensor("cosT", [D, S], BF16, kind="ExternalInput")
    sinS = nc.dram_tensor("sinS", [D, S], BF16, kind="ExternalInput")
    masks = nc.dram_tensor("masks", [4, 128, NQ], BF16, kind="ExternalInput")
    out = nc.dram_tensor("out", [S, E], BF16, kind="ExternalOutput")

    with tile.TileContext(nc) as tc:
        _emit(nc, tc, xT, wq, wk, wv, wo, cosT, sinS, masks, out)
    _split_multi_waits(nc)
    return nc


def _split_multi_waits(nc):
    """Walrus codegen only allows ONE sync-wait per TPB instruction (the
    "Too many sync wait commands" error). Tile sometimes attaches several.
    Split: insert wait-only EventSemaphore nops (one wait each) before the
    offending instruction on the same engine."""
    nsplit = 0
    for fn in nc.m.functions:
        for blk in fn.blocks:
            out_insts = []
            for inst in blk.instructions:
                si = inst.sync_info
                if si is not None and si.on_wait and len(si.on_wait) > 1:
                    waits = list(si.on_wait)
                    for k, w in enumerate(waits[:-1]):
                        ev = mybir.InstEventSemaphore(name=f"{inst.name}-ws{k}")
                        ev.engine = inst.engine
                        ev.sync_info = mybir.SyncInfo(on_wait=[w], on_update=[])
                        out_insts.append(ev)
                        nsplit += 1
                    inst.sync_info = mybir.SyncInfo(
                        on_wait=[waits[-1]], on_update=list(si.on_update or [])
                    )
                out_insts.append(inst)
            blk.instructions = out_insts
    return nsplit


PS_CFG = (2, 2, 1, 1, 2)  # PSUM pool bufs: scores, PV/V, denom, bcast, outproj


def _emit(nc, tc, xT, wq, wk, wv, wo, cosT, sinS, masks, out):
    from contextlib import ExitStack

    a, b, d, c, e = PS_CFG
    with ExitStack() as ctx:
        consts = ctx.enter_context(tc.tile_pool(name="consts", bufs=1))
        state = ctx.enter_context(tc.tile_pool(name="state", bufs=1))
        tmps = ctx.enter_context(tc.tile_pool(name="tmps", bufs=2))
        psA = ctx.enter_context(tc.tile_pool(name="psA", bufs=a, space="PSUM"))
        psB = ctx.enter_context(tc.tile_pool(name="psB", bufs=b, space="PSUM"))
        psD = ctx.enter_context(tc.tile_pool(name="psD", bufs=d, space="PSUM"))
        psC = ctx.enter_context(tc.tile_pool(name="psC", bufs=c, space="PSUM"))
        psE = ctx.enter_context(tc.tile_pool(name="psE", bufs=e, space="PSUM"))

        # ---- constants / weights to SBUF (DMA order = need order) ----
        wq_sb = consts.tile([128, EC, F2], BF16)
        wk_sb = consts.tile([128, EC, F2], BF16)
        wv_sb = consts.tile([128, EC, F2], BF16)
        wo_sb = consts.tile([128, HPC, E], BF16)
        cos_sb = consts.tile([D, S], BF16)
        sinS_sb = consts.tile([D, S], BF16)
        masks_sb = consts.tile([128, 4, NQ], BF16)
        wqr = wq.rearrange("(c p) f -> p c f", p=128)
        nc.sync.dma_start(cos_sb, cosT[:, :])
        nc.sync.dma_start(sinS_sb, sinS[:, :])
        ones_col = consts.tile([128, 1], BF16)
        nc.vector.memset(ones_col, 1.0)
        ones_colb = consts.tile([1, 128], BF16)
        nc.vector.memset(ones_colb, 1.0)

        QrT = state.tile([D, HPC, S], BF16)
        KrT = state.tile([D, HPC, S], BF16)
        V_sb = state.tile([128, NKB, F2], BF16)
        attnT = state.tile([D, HPC, S], BF16)
        eP = state.tile([128, NKB, NQ], BF16)
        ost_ring = state.tile([128, 6, NQ], BF16)
        ost_i = [0]

        if True:
            xT_sb = state.tile([128, EC, S], BF16)
            xr = xT.rearrange("(c p) s -> p c s", p=128)

            def dma_x(s4):
                for e in range(EC):
                    nc.sync.dma_start(
                        xT_sb[:, e, s4 * NQ:(s4 + 1) * NQ],
                        xr[:, e, s4 * NQ:(s4 + 1) * NQ],
                    )
            for e in range(EC):
                nc.sync.dma_start(wq_sb[:, e, :], wqr[:, e, :])
                nc.sync.dma_start(
                    xT_sb[:, e, 0:NQ],
                    xr[:, e, 0:NQ],
                )
            nc.sync.dma_start(wk_sb, wk.rearrange("(c p) f -> p c f", p=128))
            nc.sync.dma_start(wv_sb, wv.rearrange("(c p) f -> p c f", p=128))
            dma_x(1)
            nc.sync.dma_start(masks_sb, masks.rearrange("c p q -> p c q"))
            nc.sync.dma_start(wo_sb, wo.rearrange("(c p) e -> p c e", p=128))
            dma_x(2)
            dma_x(3)

            # ---- QKV projections, interleaved per s-chunk ----
            def qk_group(w_sb, dstT, f, s4):
                sl = slice(s4 * NQ, (s4 + 1) * NQ)
                ps = psA.tile([128, NQ], F32, tag="A", name="ps_proj")
                for e in range(EC):
                    nc.tensor.matmul(
                        ps,
                        lhsT=w_sb[:, e, f * 128:(f + 1) * 128],
                        rhs=xT_sb[:, e, sl],
                        start=(e == 0),
                        stop=(e == EC - 1),
                    )
                t1 = tmps.tile([128, NQ], F32, tag="ropeA", name="t1")
                t2 = tmps.tile([128, NQ], F32, tag="ropeB", name="t2")
                nc.vector.tensor_mul(t1[0:64, :], ps[64:128, :], sinS_sb[0:64, sl])
                nc.vector.tensor_mul(t1[64:128, :], ps[0:64, :], sinS_sb[64:128, sl])
                nc.vector.tensor_mul(t2, ps, cos_sb[:, sl])
                nc.vector.tensor_add(dstT[:, f, sl], t1, t2)

            def v_group(sc):
                psv = psB.tile([128, F2], F32, tag="B", name="ps_v")
                for e in range(EC):
                    nc.tensor.matmul(
                        psv,
                        lhsT=xT_sb[:, e, sc * 128:(sc + 1) * 128],
                        rhs=wv_sb[:, e, :],
                        start=(e == 0),
                        stop=(e == EC - 1),
                    )
                nc.scalar.copy(V_sb[:, sc, :], psv)

            for s4 in range(NJ):
                qk_group(wq_sb, QrT, 0, s4)
                qk_group(wk_sb, KrT, 0, s4)
                v_group(4 * s4 + 0)
                v_group(4 * s4 + 1)
                qk_group(wq_sb, QrT, 1, s4)
                qk_group(wk_sb, KrT, 1, s4)
                v_group(4 * s4 + 2)
                v_group(4 * s4 + 3)

        # ---- attention + out-proj ----
        if True:
            for j in range(NJ):
                qsl = slice(j * NQ, (j + 1) * NQ)
                nblk = 4 * (j + 1)
                for h in range(HPC):
                    ps_d = psD.tile([1, NQ], F32, tag="D", name="ps_d")
                    ps_o = psB.tile([128, NQ], F32, tag="B", name="ps_o")

                    def scores(kb):
                        ps_s = psA.tile([128, NQ], F32, tag="A", name="ps_s")
                        nc.tensor.matmul(
                            ps_s,
                            lhsT=KrT[:, h, kb * 128:(kb + 1) * 128],
                            rhs=QrT[:, h, qsl],
                            start=True,
                            stop=True,
                        )
                        nc.scalar.activation(eP[:, kb, :], ps_s, AF.Exp)
                        if kb >= nblk - 4:
                            nc.vector.tensor_mul(
                                eP[:, kb, :], eP[:, kb, :],
                                masks_sb[:, kb - (nblk - 4), :],
                            )

                    def accum(kb):
                        nc.tensor.matmul(
                            ps_d, lhsT=ones_col, rhs=eP[:, kb, :],
                            start=(kb == 0), stop=(kb == nblk - 1),
                        )
                        nc.tensor.matmul(
                            ps_o,
                            lhsT=V_sb[:, kb, h * 128:(h + 1) * 128],
                            rhs=eP[:, kb, :],
                            start=(kb == 0), stop=(kb == nblk - 1),
                        )

                    # software-pipeline depth 2: scores(kb+2) issued
                    # before accum(kb) so exp latency is hidden
                    # software-pipeline depth 2: scores(kb+2) issued
                    # before accum(kb) so exp latency is hidden
                    scores(0)
                    scores(1)
                    for kb in range(2, nblk):
                        scores(kb)
                        accum(kb - 2)
                    accum(nblk - 2)
                    accum(nblk - 1)

                    rec = tmps.tile([1, NQ], F32, tag="rec", name="rec")
                    nc.vector.reciprocal(rec, ps_d)
                    rec_hi = tmps.tile([1, NQ], BF16, tag="rech", name="rec_hi")
                    nc.vector.tensor_copy(rec_hi, rec)
                    rec_lo = tmps.tile([1, NQ], BF16, tag="recl", name="rec_lo")
                    nc.vector.tensor_sub(rec_lo, rec, rec_hi)
                    ps_b = psC.tile([128, NQ], F32, tag="C", name="ps_b")
                    nc.tensor.matmul(ps_b, lhsT=ones_colb, rhs=rec_hi, start=True, stop=False)
                    nc.tensor.matmul(ps_b, lhsT=ones_colb, rhs=rec_lo, start=False, stop=True)
                    bc = tmps.tile([128, NQ], F32, tag="bc", name="bc")
                    nc.vector.tensor_copy(bc, ps_b)
                    nc.vector.tensor_mul(attnT[:, h, qsl], ps_o, bc)

                # out-proj for this q-tile (both heads ready)
                for sc in range(4 * j, 4 * j + 4):
                    for ec in range(4):
                        pso = psE.tile([128, NQ], F32, tag="E", name="pso")
                        for hc in range(HPC):
                            nc.tensor.matmul(
                                pso,
                                lhsT=attnT[:, hc, sc * 128:(sc + 1) * 128],
                                rhs=wo_sb[:, hc, ec * NQ:(ec + 1) * NQ],
                                start=(hc == 0),
                                stop=(hc == HPC - 1),
                            )
                        oi = ost_i[0] % 6
                        ost_i[0] += 1
                        ost = ost_ring[:, oi, :]
                        # alternate the PSUM->SBUF copy between DVE and ACT
                        # so copy throughput is not the outproj bottleneck
                        if oi % 2 == 0:
                            nc.vector.tensor_copy(ost, pso)
                        else:
                            nc.scalar.copy(ost, pso)
                        nc.sync.dma_start(
                            out[sc * 128:(sc + 1) * 128, ec * NQ:(ec + 1) * NQ], ost
                        )


_NC_CACHE = None


def _get_nc():
    global _NC_CACHE
    if _NC_CACHE is None:
        _NC_CACHE = build_nc()
    return _NC_CACHE


def _prep_inputs(x, rotary_cos, rotary_sin, Wq, Wk, Wv, Wo):
    bf = ml_dtypes.bfloat16
    x = np.asarray(x, dtype=np.float32)
    Wq = np.asarray(Wq, dtype=np.float32)
    Wk = np.asarray(Wk, dtype=np.float32)
    Wv = np.asarray(Wv, dtype=np.float32)
    Wo = np.asarray(Wo, dtype=np.float32)
    cos = np.asarray(rotary_cos, dtype=np.float32)[0]  # [S, D]
    sin = np.asarray(rotary_sin, dtype=np.float32)[0]

    xT = np.ascontiguousarray(x[0].T).astype(bf)          # [E, S]
    cosT = np.ascontiguousarray(cos.T).astype(bf)          # [D, S]
    sinT = cos.T * 0 + sin.T
    sinS = np.concatenate([-sinT[:64], sinT[64:]], axis=0)
    sinS = np.ascontiguousarray(sinS).astype(bf)

    # 4 diagonal-mask tiles: mask[idx, k, q] = 1 if k + 128*idx <= q
    kk = np.arange(128)[:, None]
    qq = np.arange(NQ)[None, :]
    m = np.stack([(kk + 128 * i <= qq) for i in range(4)]).astype(bf)
    masks = np.ascontiguousarray(m)

    scale = 1.0 / math.sqrt(D)
    in_maps = []
    for c in range(NCORES):
        fs = slice(F2 * c, F2 * (c + 1))
        in_maps.append({
            "xT": xT,
            "wq": np.ascontiguousarray((Wq[fs, :] * scale).T).astype(bf),
            "wk": np.ascontiguousarray(Wk[fs, :].T).astype(bf),
            "wv": np.ascontiguousarray(Wv[fs, :].T).astype(bf),
            "wo": np.ascontiguousarray(Wo[:, fs].T).astype(bf),
            "cosT": cosT,
            "sinS": sinS,
            "masks": masks,
        })
    return in_maps


def kernel(x, rotary_cos, rotary_sin, Wq, Wk, Wv, Wo, **run_kwargs):
    nc = _get_nc()
    in_maps = _prep_inputs(x, rotary_cos, rotary_sin, Wq, Wk, Wv, Wo)
    res = run_bass_kernel_spmd(nc, in_maps, core_ids=list(range(NCORES)), **run_kwargs)
    acc = np.zeros((S, E), dtype=np.float64)
    for r in res.results:
        acc += r["out"].astype(np.float64)
    full = acc.astype(np.float32).reshape(1, S, E)
    if run_kwargs:
        return full, res
    return full
